# revision 15
# baseline (speedup 1.0000x reference)
"""BitFFN (BitNet b1.58-style quantized MLP) on 8 Trainium2 NeuronCores.

Reference computation (per-tensor int8 act quant, ternary weight quant, sub-LN):
    x_ln = layernorm(x, ln_w, ln_b)
    h    = gelu_exact( actq(x_ln) @ wq(w1).T )
    y    = actq(h) @ wq(w2).T

Key reformulation: the forward pass of both STE quantizers makes every matmul an
*integer* matmul.  qx = round(x_ln * s1) is an integer in [-127,127] (exact in
bf16); wq(w)/beta is ternary {-1,0,+1} (exact in bf16).  PSUM accumulates fp32,
and all partial sums are integers < 2^24, so the bf16 matmuls are bit-exact.
The scales fold out: h_pre = (qx @ tern1.T) * (beta1 * m1 / 127), etc.

Sharding: data-parallel over the 8192 tokens (1024 tokens/core); weights are
ternarized + packed on host and replicated.  The two *global* activation-quant
scales (per-tensor max over ALL tokens) are produced mid-kernel with a tiny
AllGather of per-core partial maxes — the only cross-core communication.

Layout strategy per core (all matmuls contract on the partition dim):
  mm1: out hT[ff,tok] : lhsT = w1T tile [d,ff] (stationary), rhs = qxT [d,tok]
  mm2: out y[tok,d]   : lhsT = qhT tile [F,tok] (stationary), rhs = w2T [F,d]
mm1's output layout IS mm2's rhs-side layout, so only qx needs an on-chip
transpose (PE-transpose of 128x128 bf16 blocks).  h is spilled to DRAM fp32
(it cannot be quantized until the global max exists) and re-quantized on reload.
"""

import sys
import numpy as np

sys.path.insert(0, "/opt/trn_rl_repo")

import ml_dtypes

import concourse.bass as bass  # noqa: F401  (import order matters)
import concourse.mybir as mybir
import concourse.tile as tile
from concourse import bacc, bass_isa
from concourse.bass_utils import run_bass_kernel_spmd
from concourse.masks import make_identity

P = 128
NCORES = 8
EPS = 1e-5
QB = 127.0
MAGIC = 1.5 * 2**23  # fp32 round-to-nearest-even magic constant

f32 = mybir.dt.float32
bf16 = mybir.dt.bfloat16
AX = mybir.AxisListType
OP = mybir.AluOpType
AF = mybir.ActivationFunctionType

# full problem dims
FULL_B, FULL_S, D_MODEL, D_FF = 4, 2048, 2048, 8192


def build(T, D, F, n_cores=NCORES, debug_taps=False, sim_gelu_sub=False):
    """Build the per-core SPMD program. T tokens/core, D=d_model, F=d_ff."""
    TT = T // P          # token tiles
    KC = D // P          # contraction chunks for mm1
    FC = F // P          # ff chunks
    TS = min(T, 512)     # mm1 psum token-split width
    NTS = T // TS        # token splits in mm1 psum
    ND2 = D // 512       # output d chunks for mm2

    nc = bacc.Bacc(
        "TRN2", target_bir_lowering=False, debug=False, enable_asserts=True,
        num_devices=n_cores,
    )
    xs = nc.dram_tensor("xs", [T, D], f32, kind="ExternalInput").ap()
    lnw = nc.dram_tensor("lnw", [D], f32, kind="ExternalInput").ap()
    lnb = nc.dram_tensor("lnb", [D], f32, kind="ExternalInput").ap()
    w1p = nc.dram_tensor("w1p", [FC, P, D], bf16, kind="ExternalInput").ap()
    w2p = nc.dram_tensor("w2p", [ND2, FC, P, 512], bf16, kind="ExternalInput").ap()
    consts = nc.dram_tensor("consts", [2], f32, kind="ExternalInput").ap()
    ys = nc.dram_tensor("ys", [T, D], f32, kind="ExternalOutput").ap()

    hsp = nc.dram_tensor("hsp", [FC, P, T], f32,
                         kind="ExternalOutput" if debug_taps else "Internal")
    if debug_taps:
        dbg_s = nc.dram_tensor("dbg_s", [P, 4], f32, kind="ExternalOutput").ap()
        dbg_xln = nc.dram_tensor("dbg_xln", [T, D], f32, kind="ExternalOutput").ap()
    ag1_in = nc.dram_tensor("ag1_in", [P], f32)
    ag1_out = nc.dram_tensor("ag1_out", [n_cores, P], f32, addr_space="Shared")
    ag2_in = nc.dram_tensor("ag2_in", [P], f32)
    ag2_out = nc.dram_tensor("ag2_out", [n_cores, P], f32, addr_space="Shared")

    rg = [list(range(n_cores))]

    with tile.TileContext(nc) as tc:
        with tc.tile_pool(name="cpool", bufs=1) as cp:
            # long-lived constants / scalars
            consts_sb = cp.tile([1, 2], f32, tag="consts")
            nc.sync.dma_start(consts_sb[:], consts)
            ident = cp.tile([P, P], f32, tag="ident")
            make_identity(nc, ident[:])

            def global_scale(acc_cols, ncols, ag_in, ag_out, const_idx):
                """per-partition-col maxes -> (sb [P,1] scale, gb [P,1] out-scale)."""
                colred = cp.tile([P, 1], f32, tag=f"colred{const_idx}")
                nc.vector.tensor_reduce(colred[:], acc_cols[:, 0:ncols], axis=AX.X,
                                        op=OP.max)
                pmax = cp.tile([P, 1], f32, tag=f"pmax{const_idx}")
                nc.gpsimd.partition_all_reduce(pmax[:], colred[:], channels=P,
                                               reduce_op=bass_isa.ReduceOp.absmax)
                nc.sync.dma_start(ag_in[:], pmax[:])
                nc.gpsimd.collective_compute(
                    "AllGather", OP.bypass, replica_groups=rg,
                    ins=[ag_in[:].opt()], outs=[ag_out[:].opt()],
                )
                agt = cp.tile([1, n_cores * P], f32, tag=f"agt{const_idx}")
                nc.sync.dma_start(agt[:], ag_out[:])
                gmax = cp.tile([1, 1], f32, tag=f"gmax{const_idx}")
                nc.vector.tensor_reduce(gmax[:], agt[:], axis=AX.X, op=OP.max)
                m = cp.tile([1, 1], f32, tag=f"m{const_idx}")
                nc.vector.tensor_scalar_max(m[:], gmax[:], EPS)
                # s = QB / m  via reciprocal + 1 Newton step
                r0 = cp.tile([1, 1], f32, tag=f"r0{const_idx}")
                nc.vector.reciprocal(r0[:], m[:])
                t0 = cp.tile([1, 1], f32, tag=f"t0{const_idx}")
                nc.vector.tensor_tensor(out=t0[:], in0=m[:], in1=r0[:], op=OP.mult)
                t1 = cp.tile([1, 1], f32, tag=f"t1{const_idx}")
                nc.vector.tensor_scalar(out=t1[:], in0=t0[:], scalar1=-1.0,
                                        scalar2=2.0, op0=OP.mult, op1=OP.add)
                r1 = cp.tile([1, 1], f32, tag=f"r1{const_idx}")
                nc.vector.tensor_tensor(out=r1[:], in0=r0[:], in1=t1[:], op=OP.mult)
                s = cp.tile([1, 1], f32, tag=f"s{const_idx}")
                nc.vector.tensor_scalar_mul(s[:], r1[:], QB)
                sb_ = cp.tile([P, 1], f32, tag=f"sb{const_idx}")
                nc.gpsimd.partition_broadcast(sb_[:], s[:], channels=P)
                # g = m * (beta/QB)
                g = cp.tile([1, 1], f32, tag=f"g{const_idx}")
                nc.vector.tensor_tensor(out=g[:], in0=m[:],
                                        in1=consts_sb[:, const_idx:const_idx + 1],
                                        op=OP.mult)
                gb = cp.tile([P, 1], f32, tag=f"gb{const_idx}")
                nc.gpsimd.partition_broadcast(gb[:], g[:], channels=P)
                return sb_, gb


            accm = cp.tile([P, TT], f32, tag="accm")
            acch = cp.tile([P, FC], f32, tag="acch")

            with tc.tile_pool(name="qxT", bufs=KC) as qxtp, \
                 tc.tile_pool(name="w1s", bufs=3) as w1sp:
                qxT = [qxtp.tile([P, T], bf16, name=f"qxT{k}", tag="qxT")
                       for k in range(KC)]
                # ---- Phase 1: LayerNorm + absmax + PE-transpose (fp32) ----
                with tc.tile_pool(name="xlT", bufs=KC) as xlTp, \
                     tc.tile_pool(name="lnp", bufs=1) as lnp:
                    lnw_sb = lnp.tile([P, D], f32, tag="lnw")
                    lnb_sb = lnp.tile([P, D], f32, tag="lnb")
                    lnw_row = lnp.tile([1, D], f32, tag="lnwr")
                    lnb_row = lnp.tile([1, D], f32, tag="lnbr")
                    nc.sync.dma_start(lnw_row[:], lnw)
                    nc.sync.dma_start(lnb_row[:], lnb)
                    nc.gpsimd.partition_broadcast(lnw_sb[:], lnw_row[:], channels=P)
                    nc.gpsimd.partition_broadcast(lnb_sb[:], lnb_row[:], channels=P)
                    xlT = [xlTp.tile([P, T], f32, name=f"xlT{k}", tag="xlT")
                           for k in range(KC)]
                    with tc.tile_pool(name="ph1", bufs=2) as ph1, \
                         tc.tile_pool(name="pstr", bufs=4, space="PSUM") as pstr:
                        for t in range(TT):
                            xt = ph1.tile([P, D], f32, tag="xt")
                            nc.sync.dma_start(xt[:], xs[t * P:(t + 1) * P, :])
                            mu = ph1.tile([P, 1], f32, tag="mu")
                            nc.vector.tensor_reduce(mu[:], xt[:], axis=AX.X,
                                                    op=OP.add)
                            nc.vector.tensor_scalar_mul(mu[:], mu[:], 1.0 / D)
                            xc = ph1.tile([P, D], f32, tag="xc")
                            nc.vector.tensor_scalar(out=xc[:], in0=xt[:],
                                                    scalar1=mu[:, 0:1],
                                                    scalar2=None, op0=OP.subtract)
                            sq = ph1.tile([P, D], f32, tag="sq")
                            nc.vector.tensor_tensor(out=sq[:], in0=xc[:],
                                                    in1=xc[:], op=OP.mult)
                            var = ph1.tile([P, 1], f32, tag="var")
                            nc.vector.tensor_reduce(var[:], sq[:], axis=AX.X,
                                                    op=OP.add)
                            a = ph1.tile([P, 1], f32, tag="a")
                            nc.vector.tensor_scalar(out=a[:], in0=var[:],
                                                    scalar1=1.0 / D, scalar2=EPS,
                                                    op0=OP.mult, op1=OP.add)
                            sq_a = ph1.tile([P, 1], f32, tag="sqa")
                            nc.scalar.sqrt(sq_a[:], a[:])
                            y0 = ph1.tile([P, 1], f32, tag="y0")
                            nc.vector.reciprocal(y0[:], sq_a[:])
                            # Newton rsqrt: y1 = y0*(1.5 - 0.5*a*y0^2)
                            n0 = ph1.tile([P, 1], f32, tag="n0")
                            nc.vector.tensor_tensor(out=n0[:], in0=y0[:],
                                                    in1=y0[:], op=OP.mult)
                            n1 = ph1.tile([P, 1], f32, tag="n1")
                            nc.vector.tensor_tensor(out=n1[:], in0=a[:],
                                                    in1=n0[:], op=OP.mult)
                            n2 = ph1.tile([P, 1], f32, tag="n2")
                            nc.vector.tensor_scalar(out=n2[:], in0=n1[:],
                                                    scalar1=-0.5, scalar2=1.5,
                                                    op0=OP.mult, op1=OP.add)
                            rstd = ph1.tile([P, 1], f32, tag="rstd")
                            nc.vector.tensor_tensor(out=rstd[:], in0=y0[:],
                                                    in1=n2[:], op=OP.mult)
                            # in-place: xc = ((xc*rstd) * ln_w) + ln_b = x_ln
                            nc.vector.tensor_scalar(out=xc[:], in0=xc[:],
                                                    scalar1=rstd[:, 0:1],
                                                    scalar2=None, op0=OP.mult)
                            nc.vector.tensor_tensor(out=xc[:], in0=xc[:],
                                                    in1=lnw_sb[:], op=OP.mult)
                            nc.vector.tensor_tensor(out=xc[:], in0=xc[:],
                                                    in1=lnb_sb[:], op=OP.add)
                            nc.vector.tensor_reduce(accm[:, t:t + 1], xc[:],
                                                    axis=AX.X, op=OP.max,
                                                    apply_absolute_value=True)
                            if debug_taps:
                                nc.sync.dma_start(dbg_xln[t * P:(t + 1) * P, :],
                                                  xc[:])
                            # PE-transpose x_ln into [d, tok] layout (keeps PE
                            # warm through the LN phase)
                            for k in range(KC):
                                ptr = pstr.tile([P, P], f32,
                                                name=f"ptr{t}_{k}", tag="ptr")
                                nc.tensor.transpose(ptr[:],
                                                    xc[:, k * P:(k + 1) * P],
                                                    ident[:])
                                nc.vector.tensor_copy(
                                    xlT[k][:, t * P:(t + 1) * P], ptr[:])

                    # prefetch first w1 chunks before the Sync engine stalls
                    # on the AllGather-dependent loads
                    wf_pre = []
                    for f in range(min(3, FC)):
                        wf = w1sp.tile([P, D], bf16, name=f"wfp{f}", tag="wf")
                        nc.sync.dma_start(wf[:], w1p[f])
                        wf_pre.append(wf)

                    s1b, g1b = global_scale(accm, TT, ag1_in, ag1_out, 0)

                    # quantize in transposed layout (in-place magic rounding)
                    for k in range(KC):
                        nc.vector.tensor_scalar(out=xlT[k][:], in0=xlT[k][:],
                                                scalar1=s1b[:, 0:1],
                                                scalar2=MAGIC,
                                                op0=OP.mult, op1=OP.add)
                        nc.vector.tensor_scalar(out=qxT[k][:], in0=xlT[k][:],
                                                scalar1=MAGIC, scalar2=None,
                                                op0=OP.subtract)

                # --------------------- Phase 2b: mm1 ----------------------
                with tc.tile_pool(name="hT", bufs=3) as hTp, \
                     tc.tile_pool(name="ps1", bufs=3, space="PSUM") as ps1p:
                    for f in range(FC):
                        if f < len(wf_pre):
                            wf = wf_pre[f]
                        else:
                            wf = w1sp.tile([P, D], bf16, name=f"wf{f}", tag="wf")
                            nc.sync.dma_start(wf[:], w1p[f])
                        ps = ps1p.tile([P, T], f32, tag="ps")
                        for k in range(KC):
                            lw = wf[:, k * P:(k + 1) * P]
                            for j in range(NTS):
                                nc.tensor.matmul(
                                    ps[:, j * TS:(j + 1) * TS], lw,
                                    qxT[k][:, j * TS:(j + 1) * TS],
                                    start=(k == 0), stop=(k == KC - 1))
                        # DVE drains PSUM exactly (ACT PSUM-read is lossy)
                        hpre = hTp.tile([P, T], f32, tag="hpre")
                        for j in range(NTS):
                            nc.vector.tensor_scalar(
                                out=hpre[:, j * TS:(j + 1) * TS],
                                in0=ps[:, j * TS:(j + 1) * TS],
                                scalar1=g1b[:, 0:1], scalar2=None,
                                op0=OP.mult)
                        # gelu_exact(x) = x * (0.5 + 0.5*erf(x/sqrt(2)))
                        # (composed via Erf: the Gelu LUT clamps to identity in
                        # the tail, Erf saturates to 1 with tiny error)
                        ge = hTp.tile([P, T], f32, tag="ge")
                        if sim_gelu_sub:
                            nc.scalar.activation(ge[:], hpre[:], AF.Tanh,
                                                 scale=0.7071067811865476)
                        else:
                            nc.scalar.activation(ge[:], hpre[:], AF.Erf,
                                                 scale=0.7071067811865476)
                        nc.vector.tensor_scalar(out=ge[:], in0=ge[:],
                                                scalar1=0.5, scalar2=0.5,
                                                op0=OP.mult, op1=OP.add)
                        hT = hTp.tile([P, T], f32, tag="hT")
                        nc.vector.tensor_tensor(out=hT[:], in0=hpre[:],
                                                in1=ge[:], op=OP.mult)
                        nc.vector.tensor_reduce(acch[:, f:f + 1], hT[:],
                                                axis=AX.X, op=OP.max,
                                                apply_absolute_value=True)
                        nc.sync.dma_start(hsp[f], hT[:])

            # ---------------- Phase 3: requantize h + mm2 ----------------
            NPRE = min(8, FC)
            with tc.tile_pool(name="qhT", bufs=FC) as qhp, \
                 tc.tile_pool(name="rel", bufs=1) as relp, \
                 tc.tile_pool(name="w2s", bufs=3) as w2sp, \
                 tc.tile_pool(name="yst", bufs=3) as ysp, \
                 tc.tile_pool(name="ps2", bufs=8, space="PSUM") as ps2p:
                # prefetch reloads of h before the AllGather-dependent DMAs
                # enter the in-order Sync stream
                hr_tiles = {}
                for f in range(NPRE):
                    hr = relp.tile([P, T], f32, name=f"hr{f}", tag="hr",
                                   bufs=NPRE)
                    nc.sync.dma_start(hr[:], hsp[f])
                    hr_tiles[f] = hr

                s2b, g2b = global_scale(acch, FC, ag2_in, ag2_out, 1)

                qhT = []
                for f in range(FC):
                    if f in hr_tiles:
                        hr = hr_tiles[f]
                    else:
                        hr = relp.tile([P, T], f32, name=f"hr{f}", tag="hr",
                                       bufs=NPRE)
                        nc.sync.dma_start(hr[:], hsp[f])
                    # in-place magic rounding: hr = hr*s2 + MAGIC; qh = hr-MAGIC
                    nc.vector.tensor_scalar(out=hr[:], in0=hr[:],
                                            scalar1=s2b[:, 0:1], scalar2=MAGIC,
                                            op0=OP.mult, op1=OP.add)
                    qh = qhp.tile([P, T], bf16, name=f"qhT{f}", tag="qhT")
                    nc.vector.tensor_scalar(out=qh[:], in0=hr[:], scalar1=MAGIC,
                                            scalar2=None, op0=OP.subtract)
                    qhT.append(qh)
                for n in range(ND2):
                    pss = [ps2p.tile([P, 512], f32, name=f"psy{n}_{t}", tag="psy")
                           for t in range(TT)]
                    for f in range(FC):
                        w2t = w2sp.tile([P, 512], bf16, tag="w2t")
                        nc.sync.dma_start(w2t[:], w2p[n, f])
                        for t in range(TT):
                            nc.tensor.matmul(pss[t][:],
                                             qhT[f][:, t * P:(t + 1) * P],
                                             w2t[:], start=(f == 0),
                                             stop=(f == FC - 1))
                    for t in range(TT):
                        yt = ysp.tile([P, 512], f32, tag="yt")
                        nc.vector.tensor_scalar(out=yt[:], in0=pss[t][:],
                                                scalar1=g2b[:, 0:1], scalar2=None,
                                                op0=OP.mult)
                        nc.sync.dma_start(
                            ys[t * P:(t + 1) * P, n * 512:(n + 1) * 512], yt[:])

    nc.compile()
    return nc


_CACHE = {}


def _get_program(T, D, F, n_cores=NCORES):
    key = (T, D, F, n_cores)
    if key not in _CACHE:
        _CACHE[key] = build(T, D, F, n_cores)
    return _CACHE[key]


def _prep_host(x, ln_w, ln_b, w1, w2, n_cores=NCORES):
    """Host-side prep: shard x, ternarize+pack weights, compute consts."""
    B, S, D = x.shape
    F = w1.shape[0]
    T = B * S // n_cores
    ND2 = D // 512
    FC = F // P

    x2 = np.ascontiguousarray(x.reshape(B * S, D).astype(np.float32, copy=False))

    def tern(w):
        beta = np.float32(max(np.abs(w).mean(dtype=np.float64), EPS))
        q = np.clip(np.round(w / beta), -1.0, 1.0).astype(np.float32)
        return q, beta

    w1t, b1 = tern(np.asarray(w1, np.float32))   # [F, D]
    w2t, b2 = tern(np.asarray(w2, np.float32))   # [D, F]

    # w1p[f, p, k*P+m'] wait -- see build: w1p is [FC, P, D]: w1p[f, p, j] with
    # j = k*P + m NO: lhsT tile for (f,k) is wf[:, k*P:(k+1)*P] = [d-part p, ff m]
    # wf[p, k*P+m] must equal w1T[k*P+p, f*P+m] = w1t[f*P+m, k*P+p]
    a = w1t.reshape(FC, P, D // P, P)           # [f, m, k, p]
    w1p = np.ascontiguousarray(a.transpose(0, 3, 2, 1).reshape(FC, P, D)
                               ).astype(ml_dtypes.bfloat16)
    # w2p[n, f, p, j] = w2T[f*P+p, n*512+j] = w2t[n*512+j, f*P+p]
    b_ = w2t.reshape(ND2, 512, FC, P)           # [n, j, f, p]
    w2p = np.ascontiguousarray(b_.transpose(0, 2, 3, 1)).astype(ml_dtypes.bfloat16)

    consts = np.array([b1 / np.float32(QB), b2 / np.float32(QB)], dtype=np.float32)
    lnw = np.ascontiguousarray(np.asarray(ln_w, np.float32))
    lnb = np.ascontiguousarray(np.asarray(ln_b, np.float32))

    in_maps = []
    for c in range(n_cores):
        in_maps.append({
            "xs": np.ascontiguousarray(x2[c * T:(c + 1) * T]),
            "lnw": lnw, "lnb": lnb,
            "w1p": w1p, "w2p": w2p, "consts": consts,
        })
    return in_maps, T


def kernel(x, ln_w, ln_b, w1, w2):
    x = np.asarray(x)
    B, S, D = x.shape
    F = np.asarray(w1).shape[0]
    in_maps, T = _prep_host(x, ln_w, ln_b, w1, w2)
    nc = _get_program(T, D, F)
    res = run_bass_kernel_spmd(nc, in_maps, list(range(NCORES)))
    y = np.concatenate([res.results[c]["ys"] for c in range(NCORES)], axis=0)
    return np.ascontiguousarray(y.reshape(B, S, D).astype(np.float32))


# revision 17
# speedup vs baseline: 1.1072x; 1.1072x over previous
"""BitFFN (BitNet b1.58-style quantized MLP) on 8 Trainium2 NeuronCores.

Reference computation (per-tensor int8 act quant, ternary weight quant, sub-LN):
    x_ln = layernorm(x, ln_w, ln_b)
    h    = gelu_exact( actq(x_ln) @ wq(w1).T )
    y    = actq(h) @ wq(w2).T

Key reformulation: the forward pass of both STE quantizers makes every matmul an
*integer* matmul.  qx = round(x_ln * s1) is an integer in [-127,127] (exact in
bf16); wq(w)/beta is ternary {-1,0,+1} (exact in bf16).  PSUM accumulates fp32,
and all partial sums are integers < 2^24, so the bf16 matmuls are bit-exact.
The scales fold out: h_pre = (qx @ tern1.T) * (beta1 * m1 / 127), etc.

Sharding: data-parallel over the 8192 tokens (1024 tokens/core); weights are
ternarized + packed on host and replicated.  The two *global* activation-quant
scales (per-tensor max over ALL tokens) are produced mid-kernel with a tiny
AllGather of per-core partial maxes — the only cross-core communication.

Layout strategy per core (all matmuls contract on the partition dim):
  mm1: out hT[ff,tok] : lhsT = w1T tile [d,ff] (stationary), rhs = qxT [d,tok]
  mm2: out y[tok,d]   : lhsT = qhT tile [F,tok] (stationary), rhs = w2T [F,d]
mm1's output layout IS mm2's rhs-side layout, so only qx needs an on-chip
transpose (PE-transpose of 128x128 bf16 blocks).  h is spilled to DRAM fp32
(it cannot be quantized until the global max exists) and re-quantized on reload.
"""

import sys
import numpy as np

sys.path.insert(0, "/opt/trn_rl_repo")

import ml_dtypes

import concourse.bass as bass  # noqa: F401  (import order matters)
import concourse.mybir as mybir
import concourse.tile as tile
from concourse import bacc, bass_isa
from concourse.bass_utils import run_bass_kernel_spmd
from concourse.masks import make_identity

P = 128
NCORES = 8
EPS = 1e-5
QB = 127.0
MAGIC = 1.5 * 2**23  # fp32 round-to-nearest-even magic constant

f32 = mybir.dt.float32
bf16 = mybir.dt.bfloat16
AX = mybir.AxisListType
OP = mybir.AluOpType
AF = mybir.ActivationFunctionType

# full problem dims
FULL_B, FULL_S, D_MODEL, D_FF = 4, 2048, 2048, 8192


def build(T, D, F, n_cores=NCORES, debug_taps=False, sim_gelu_sub=False):
    """Build the per-core SPMD program. T tokens/core, D=d_model, F=d_ff."""
    TT = T // P          # token tiles
    KC = D // P          # contraction chunks for mm1
    FC = F // P          # ff chunks
    TS = min(T, 512)     # mm1 psum token-split width
    NTS = T // TS        # token splits in mm1 psum
    ND2 = D // 512       # output d chunks for mm2

    nc = bacc.Bacc(
        "TRN2", target_bir_lowering=False, debug=False, enable_asserts=True,
        num_devices=n_cores,
    )
    xs = nc.dram_tensor("xs", [T, D], f32, kind="ExternalInput").ap()
    lnw = nc.dram_tensor("lnw", [D], f32, kind="ExternalInput").ap()
    lnb = nc.dram_tensor("lnb", [D], f32, kind="ExternalInput").ap()
    w1p = nc.dram_tensor("w1p", [FC, P, D], bf16, kind="ExternalInput").ap()
    w2p = nc.dram_tensor("w2p", [ND2, FC, P, 512], bf16, kind="ExternalInput").ap()
    consts = nc.dram_tensor("consts", [2], f32, kind="ExternalInput").ap()
    ys = nc.dram_tensor("ys", [T, D], f32, kind="ExternalOutput").ap()

    hsp = nc.dram_tensor("hsp", [FC, P, T], f32,
                         kind="ExternalOutput" if debug_taps else "Internal")
    if debug_taps:
        dbg_s = nc.dram_tensor("dbg_s", [P, 4], f32, kind="ExternalOutput").ap()
        dbg_xln = nc.dram_tensor("dbg_xln", [T, D], f32, kind="ExternalOutput").ap()
    ag1_in = nc.dram_tensor("ag1_in", [P], f32)
    ag1_out = nc.dram_tensor("ag1_out", [n_cores, P], f32, addr_space="Shared")
    ag2_in = nc.dram_tensor("ag2_in", [P], f32)
    ag2_out = nc.dram_tensor("ag2_out", [n_cores, P], f32, addr_space="Shared")

    rg = [list(range(n_cores))]

    with tile.TileContext(nc) as tc:
        with tc.tile_pool(name="cpool", bufs=1) as cp:
            # long-lived constants / scalars
            consts_sb = cp.tile([1, 2], f32, tag="consts")
            nc.sync.dma_start(consts_sb[:], consts)
            ident = cp.tile([P, P], f32, tag="ident")
            make_identity(nc, ident[:])
            mgc = cp.tile([P, 1], f32, tag="mgc")
            nc.gpsimd.memset(mgc[:], MAGIC)
            nmgc = cp.tile([P, 1], f32, tag="nmgc")
            nc.gpsimd.memset(nmgc[:], -MAGIC)

            def global_scale(acc_cols, ncols, ag_in, ag_out, const_idx):
                """per-partition-col maxes -> (sb [P,1] scale, gb [P,1] out-scale)."""
                colred = cp.tile([P, 1], f32, tag=f"colred{const_idx}")
                nc.vector.tensor_reduce(colred[:], acc_cols[:, 0:ncols], axis=AX.X,
                                        op=OP.max)
                pmax = cp.tile([P, 1], f32, tag=f"pmax{const_idx}")
                nc.gpsimd.partition_all_reduce(pmax[:], colred[:], channels=P,
                                               reduce_op=bass_isa.ReduceOp.absmax)
                nc.sync.dma_start(ag_in[:], pmax[:])
                nc.gpsimd.collective_compute(
                    "AllGather", OP.bypass, replica_groups=rg,
                    ins=[ag_in[:].opt()], outs=[ag_out[:].opt()],
                )
                agt = cp.tile([1, n_cores * P], f32, tag=f"agt{const_idx}")
                nc.sync.dma_start(agt[:], ag_out[:])
                gmax = cp.tile([1, 1], f32, tag=f"gmax{const_idx}")
                nc.vector.tensor_reduce(gmax[:], agt[:], axis=AX.X, op=OP.max)
                m = cp.tile([1, 1], f32, tag=f"m{const_idx}")
                nc.vector.tensor_scalar_max(m[:], gmax[:], EPS)
                # s = QB / m  via reciprocal + 1 Newton step
                r0 = cp.tile([1, 1], f32, tag=f"r0{const_idx}")
                nc.vector.reciprocal(r0[:], m[:])
                t0 = cp.tile([1, 1], f32, tag=f"t0{const_idx}")
                nc.vector.tensor_tensor(out=t0[:], in0=m[:], in1=r0[:], op=OP.mult)
                t1 = cp.tile([1, 1], f32, tag=f"t1{const_idx}")
                nc.vector.tensor_scalar(out=t1[:], in0=t0[:], scalar1=-1.0,
                                        scalar2=2.0, op0=OP.mult, op1=OP.add)
                r1 = cp.tile([1, 1], f32, tag=f"r1{const_idx}")
                nc.vector.tensor_tensor(out=r1[:], in0=r0[:], in1=t1[:], op=OP.mult)
                s = cp.tile([1, 1], f32, tag=f"s{const_idx}")
                nc.vector.tensor_scalar_mul(s[:], r1[:], QB)
                sb_ = cp.tile([P, 1], f32, tag=f"sb{const_idx}")
                nc.gpsimd.partition_broadcast(sb_[:], s[:], channels=P)
                # g = m * (beta/QB)
                g = cp.tile([1, 1], f32, tag=f"g{const_idx}")
                nc.vector.tensor_tensor(out=g[:], in0=m[:],
                                        in1=consts_sb[:, const_idx:const_idx + 1],
                                        op=OP.mult)
                gb = cp.tile([P, 1], f32, tag=f"gb{const_idx}")
                nc.gpsimd.partition_broadcast(gb[:], g[:], channels=P)
                return sb_, gb


            accm = cp.tile([P, TT], f32, tag="accm")
            acch = cp.tile([P, FC], f32, tag="acch")

            with tc.tile_pool(name="qxT", bufs=KC) as qxtp, \
                 tc.tile_pool(name="w1s", bufs=3) as w1sp:
                qxT = [qxtp.tile([P, T], bf16, name=f"qxT{k}", tag="qxT")
                       for k in range(KC)]
                # ---- Phase 1: LayerNorm + absmax + PE-transpose (fp32) ----
                with tc.tile_pool(name="xlT", bufs=KC) as xlTp, \
                     tc.tile_pool(name="lnp", bufs=1) as lnp:
                    lnw_sb = lnp.tile([P, D], f32, tag="lnw")
                    lnb_sb = lnp.tile([P, D], f32, tag="lnb")
                    lnw_row = lnp.tile([1, D], f32, tag="lnwr")
                    lnb_row = lnp.tile([1, D], f32, tag="lnbr")
                    nc.sync.dma_start(lnw_row[:], lnw)
                    nc.sync.dma_start(lnb_row[:], lnb)
                    nc.gpsimd.partition_broadcast(lnw_sb[:], lnw_row[:], channels=P)
                    nc.gpsimd.partition_broadcast(lnb_sb[:], lnb_row[:], channels=P)
                    xlT = [xlTp.tile([P, T], f32, name=f"xlT{k}", tag="xlT")
                           for k in range(KC)]
                    with tc.tile_pool(name="ph1", bufs=2) as ph1, \
                         tc.tile_pool(name="pstr", bufs=4, space="PSUM") as pstr:
                        for t in range(TT):
                            xt = ph1.tile([P, D], f32, tag="xt")
                            nc.sync.dma_start(xt[:], xs[t * P:(t + 1) * P, :])
                            mu = ph1.tile([P, 1], f32, tag="mu")
                            nc.vector.tensor_reduce(mu[:], xt[:], axis=AX.X,
                                                    op=OP.add)
                            nc.vector.tensor_scalar_mul(mu[:], mu[:], 1.0 / D)
                            xc = ph1.tile([P, D], f32, tag="xc")
                            nc.vector.tensor_scalar(out=xc[:], in0=xt[:],
                                                    scalar1=mu[:, 0:1],
                                                    scalar2=None, op0=OP.subtract)
                            sq = ph1.tile([P, D], f32, tag="sq")
                            nc.vector.tensor_tensor(out=sq[:], in0=xc[:],
                                                    in1=xc[:], op=OP.mult)
                            var = ph1.tile([P, 1], f32, tag="var")
                            nc.vector.tensor_reduce(var[:], sq[:], axis=AX.X,
                                                    op=OP.add)
                            a = ph1.tile([P, 1], f32, tag="a")
                            nc.vector.tensor_scalar(out=a[:], in0=var[:],
                                                    scalar1=1.0 / D, scalar2=EPS,
                                                    op0=OP.mult, op1=OP.add)
                            sq_a = ph1.tile([P, 1], f32, tag="sqa")
                            nc.scalar.sqrt(sq_a[:], a[:])
                            y0 = ph1.tile([P, 1], f32, tag="y0")
                            nc.vector.reciprocal(y0[:], sq_a[:])
                            # Newton rsqrt: y1 = y0*(1.5 - 0.5*a*y0^2)
                            n0 = ph1.tile([P, 1], f32, tag="n0")
                            nc.vector.tensor_tensor(out=n0[:], in0=y0[:],
                                                    in1=y0[:], op=OP.mult)
                            n1 = ph1.tile([P, 1], f32, tag="n1")
                            nc.vector.tensor_tensor(out=n1[:], in0=a[:],
                                                    in1=n0[:], op=OP.mult)
                            n2 = ph1.tile([P, 1], f32, tag="n2")
                            nc.vector.tensor_scalar(out=n2[:], in0=n1[:],
                                                    scalar1=-0.5, scalar2=1.5,
                                                    op0=OP.mult, op1=OP.add)
                            rstd = ph1.tile([P, 1], f32, tag="rstd")
                            nc.vector.tensor_tensor(out=rstd[:], in0=y0[:],
                                                    in1=n2[:], op=OP.mult)
                            # xc = x*rstd - mu*rstd = (x-mu)*rstd (one pass)
                            mr = ph1.tile([P, 1], f32, tag="mr")
                            nc.vector.tensor_tensor(out=mr[:], in0=mu[:],
                                                    in1=rstd[:], op=OP.mult)
                            nc.vector.tensor_scalar(out=xc[:], in0=xt[:],
                                                    scalar1=rstd[:, 0:1],
                                                    scalar2=mr[:, 0:1],
                                                    op0=OP.mult, op1=OP.subtract)
                            nc.vector.tensor_tensor(out=xc[:], in0=xc[:],
                                                    in1=lnw_sb[:], op=OP.mult)
                            nc.vector.tensor_tensor(out=xc[:], in0=xc[:],
                                                    in1=lnb_sb[:], op=OP.add)
                            nc.vector.tensor_reduce(accm[:, t:t + 1], xc[:],
                                                    axis=AX.X, op=OP.max,
                                                    apply_absolute_value=True)
                            if debug_taps:
                                nc.sync.dma_start(dbg_xln[t * P:(t + 1) * P, :],
                                                  xc[:])
                            # PE-transpose x_ln into [d, tok] layout (keeps PE
                            # warm through the LN phase)
                            for k in range(KC):
                                ptr = pstr.tile([P, P], f32,
                                                name=f"ptr{t}_{k}", tag="ptr")
                                nc.tensor.transpose(ptr[:],
                                                    xc[:, k * P:(k + 1) * P],
                                                    ident[:])
                                nc.vector.tensor_copy(
                                    xlT[k][:, t * P:(t + 1) * P], ptr[:])

                    # prefetch first w1 chunks before the Sync engine stalls
                    # on the AllGather-dependent loads
                    wf_pre = []
                    for f in range(min(3, FC)):
                        wf = w1sp.tile([P, D], bf16, name=f"wfp{f}", tag="wf")
                        nc.sync.dma_start(wf[:], w1p[f])
                        wf_pre.append(wf)

                    s1b, g1b = global_scale(accm, TT, ag1_in, ag1_out, 0)

                    # quantize in transposed layout (in-place magic
                    # rounding) on ACT -- validated bit-exact vs DVE; keeps
                    # DVE free and ACT is otherwise idle in this window
                    for k in range(KC):
                        nc.scalar.activation(xlT[k][:], xlT[k][:], AF.Identity,
                                             scale=s1b[:, 0:1],
                                             bias=mgc[:, 0:1])
                        nc.scalar.activation(qxT[k][:], xlT[k][:], AF.Identity,
                                             bias=nmgc[:, 0:1])

                # --------------------- Phase 2b: mm1 ----------------------
                with tc.tile_pool(name="hT", bufs=3) as hTp, \
                     tc.tile_pool(name="ps1", bufs=3, space="PSUM") as ps1p:
                    for f in range(FC):
                        if f < len(wf_pre):
                            wf = wf_pre[f]
                        else:
                            wf = w1sp.tile([P, D], bf16, name=f"wf{f}", tag="wf")
                            nc.sync.dma_start(wf[:], w1p[f])
                        ps = ps1p.tile([P, T], f32, tag="ps")
                        for k in range(KC):
                            lw = wf[:, k * P:(k + 1) * P]
                            for j in range(NTS):
                                nc.tensor.matmul(
                                    ps[:, j * TS:(j + 1) * TS], lw,
                                    qxT[k][:, j * TS:(j + 1) * TS],
                                    start=(k == 0), stop=(k == KC - 1))
                        # DVE drains PSUM exactly (ACT PSUM-read is lossy)
                        hpre = hTp.tile([P, T], f32, tag="hpre")
                        for j in range(NTS):
                            nc.vector.tensor_scalar(
                                out=hpre[:, j * TS:(j + 1) * TS],
                                in0=ps[:, j * TS:(j + 1) * TS],
                                scalar1=g1b[:, 0:1], scalar2=None,
                                op0=OP.mult)
                        # gelu_exact(x) = x * (0.5 + 0.5*erf(x/sqrt(2)))
                        # (composed via Erf: the Gelu LUT clamps to identity in
                        # the tail, Erf saturates to 1 with tiny error)
                        ge = hTp.tile([P, T], f32, tag="ge")
                        if sim_gelu_sub:
                            nc.scalar.activation(ge[:], hpre[:], AF.Tanh,
                                                 scale=0.7071067811865476)
                        else:
                            nc.scalar.activation(ge[:], hpre[:], AF.Erf,
                                                 scale=0.7071067811865476)
                        nc.vector.tensor_scalar(out=ge[:], in0=ge[:],
                                                scalar1=0.5, scalar2=0.5,
                                                op0=OP.mult, op1=OP.add)
                        hT = hTp.tile([P, T], f32, tag="hT")
                        nc.vector.tensor_tensor(out=hT[:], in0=hpre[:],
                                                in1=ge[:], op=OP.mult)
                        nc.vector.tensor_reduce(acch[:, f:f + 1], hT[:],
                                                axis=AX.X, op=OP.max,
                                                apply_absolute_value=True)
                        nc.sync.dma_start(hsp[f], hT[:])

            # ---------------- Phase 3: requantize h + mm2 ----------------
            # mm2's n=0 pass is fused with the reload+quantize loop so the
            # h-reload and w2-tile DMAs interleave in the in-order Sync
            # stream (otherwise the first w2 tile queues behind all 64
            # h reloads and the PE stalls ~100us).
            NPRE = min(8, FC)
            WPRE = min(12, FC)
            with tc.tile_pool(name="qhT", bufs=FC) as qhp, \
                 tc.tile_pool(name="rel", bufs=1) as relp, \
                 tc.tile_pool(name="w2s", bufs=16) as w2sp, \
                 tc.tile_pool(name="yst", bufs=3) as ysp, \
                 tc.tile_pool(name="ps2", bufs=8, space="PSUM") as ps2p:
                # prefetches issued before the AllGather-dependent DMAs
                hr_tiles = {}
                for f in range(NPRE):
                    hr = relp.tile([P, T], f32, name=f"hr{f}", tag="hr",
                                   bufs=NPRE)
                    nc.sync.dma_start(hr[:], hsp[f])
                    hr_tiles[f] = hr
                w2_pre = {}
                for f in range(WPRE):
                    w2t = w2sp.tile([P, 512], bf16, name=f"w2p0_{f}", tag="w2t")
                    nc.sync.dma_start(w2t[:], w2p[0, f])
                    w2_pre[f] = w2t

                s2b, g2b = global_scale(acch, FC, ag2_in, ag2_out, 1)

                def drain_psums(pss, n):
                    for t in range(TT):
                        yt = ysp.tile([P, 512], f32, name=f"yt{n}_{t}", tag="yt")
                        nc.vector.tensor_scalar(out=yt[:], in0=pss[t][:],
                                                scalar1=g2b[:, 0:1],
                                                scalar2=None, op0=OP.mult)
                        nc.sync.dma_start(
                            ys[t * P:(t + 1) * P, n * 512:(n + 1) * 512], yt[:])

                qhT = []
                pss0 = [ps2p.tile([P, 512], f32, name=f"psy0_{t}", tag="psy")
                        for t in range(TT)]
                for f in range(FC):
                    if f in hr_tiles:
                        hr = hr_tiles[f]
                    else:
                        hr = relp.tile([P, T], f32, name=f"hr{f}", tag="hr",
                                       bufs=NPRE)
                        nc.sync.dma_start(hr[:], hsp[f])
                    # in-place magic rounding: hr = hr*s2 + MAGIC; qh = hr-MAGIC
                    nc.vector.tensor_scalar(out=hr[:], in0=hr[:],
                                            scalar1=s2b[:, 0:1], scalar2=MAGIC,
                                            op0=OP.mult, op1=OP.add)
                    qh = qhp.tile([P, T], bf16, name=f"qhT{f}", tag="qhT")
                    nc.vector.tensor_scalar(out=qh[:], in0=hr[:], scalar1=MAGIC,
                                            scalar2=None, op0=OP.subtract)
                    qhT.append(qh)
                    if f in w2_pre:
                        w2t = w2_pre[f]
                    else:
                        w2t = w2sp.tile([P, 512], bf16, name=f"w2p0_{f}",
                                        tag="w2t")
                        nc.sync.dma_start(w2t[:], w2p[0, f])
                    for t in range(TT):
                        nc.tensor.matmul(pss0[t][:],
                                         qh[:, t * P:(t + 1) * P],
                                         w2t[:], start=(f == 0),
                                         stop=(f == FC - 1))
                drain_psums(pss0, 0)
                for n in range(1, ND2):
                    pss = [ps2p.tile([P, 512], f32, name=f"psy{n}_{t}",
                                     tag="psy") for t in range(TT)]
                    for f in range(FC):
                        w2t = w2sp.tile([P, 512], bf16, name=f"w2{n}_{f}",
                                        tag="w2t")
                        nc.sync.dma_start(w2t[:], w2p[n, f])
                        for t in range(TT):
                            nc.tensor.matmul(pss[t][:],
                                             qhT[f][:, t * P:(t + 1) * P],
                                             w2t[:], start=(f == 0),
                                             stop=(f == FC - 1))
                    drain_psums(pss, n)
    nc.compile()
    return nc


_CACHE = {}


def _get_program(T, D, F, n_cores=NCORES):
    key = (T, D, F, n_cores)
    if key not in _CACHE:
        _CACHE[key] = build(T, D, F, n_cores)
    return _CACHE[key]


def _prep_host(x, ln_w, ln_b, w1, w2, n_cores=NCORES):
    """Host-side prep: shard x, ternarize+pack weights, compute consts."""
    B, S, D = x.shape
    F = w1.shape[0]
    T = B * S // n_cores
    ND2 = D // 512
    FC = F // P

    x2 = np.ascontiguousarray(x.reshape(B * S, D).astype(np.float32, copy=False))

    def tern(w):
        beta = np.float32(max(np.abs(w).mean(dtype=np.float64), EPS))
        q = np.clip(np.round(w / beta), -1.0, 1.0).astype(np.float32)
        return q, beta

    w1t, b1 = tern(np.asarray(w1, np.float32))   # [F, D]
    w2t, b2 = tern(np.asarray(w2, np.float32))   # [D, F]

    # w1p[f, p, k*P+m'] wait -- see build: w1p is [FC, P, D]: w1p[f, p, j] with
    # j = k*P + m NO: lhsT tile for (f,k) is wf[:, k*P:(k+1)*P] = [d-part p, ff m]
    # wf[p, k*P+m] must equal w1T[k*P+p, f*P+m] = w1t[f*P+m, k*P+p]
    a = w1t.reshape(FC, P, D // P, P)           # [f, m, k, p]
    w1p = np.ascontiguousarray(a.transpose(0, 3, 2, 1).reshape(FC, P, D)
                               ).astype(ml_dtypes.bfloat16)
    # w2p[n, f, p, j] = w2T[f*P+p, n*512+j] = w2t[n*512+j, f*P+p]
    b_ = w2t.reshape(ND2, 512, FC, P)           # [n, j, f, p]
    w2p = np.ascontiguousarray(b_.transpose(0, 2, 3, 1)).astype(ml_dtypes.bfloat16)

    consts = np.array([b1 / np.float32(QB), b2 / np.float32(QB)], dtype=np.float32)
    lnw = np.ascontiguousarray(np.asarray(ln_w, np.float32))
    lnb = np.ascontiguousarray(np.asarray(ln_b, np.float32))

    in_maps = []
    for c in range(n_cores):
        in_maps.append({
            "xs": np.ascontiguousarray(x2[c * T:(c + 1) * T]),
            "lnw": lnw, "lnb": lnb,
            "w1p": w1p, "w2p": w2p, "consts": consts,
        })
    return in_maps, T


def kernel(x, ln_w, ln_b, w1, w2):
    x = np.asarray(x)
    B, S, D = x.shape
    F = np.asarray(w1).shape[0]
    in_maps, T = _prep_host(x, ln_w, ln_b, w1, w2)
    nc = _get_program(T, D, F)
    res = run_bass_kernel_spmd(nc, in_maps, list(range(NCORES)))
    y = np.concatenate([res.results[c]["ys"] for c in range(NCORES)], axis=0)
    return np.ascontiguousarray(y.reshape(B, S, D).astype(np.float32))
